# revision 1
# baseline (speedup 1.0000x reference)
"""Canny NMS filter for 8x Trainium2 NeuronCores (Bass/Tile).

Batch is sharded one image per core. Per core (img 3x1024x1024 -> 1024x1024):

  m   = (c0+c1+c2)           DMA accumulate loads (SWDGE), fp32
  mb  = gauss3x3 (x) pad(m)  banded bf16 hi/lo-pair matmuls on PE
  gx  = sobelx (x) pad(mb)   banded bf16-pair matmuls (weights exact bf16)
  gy  = sobely (x) pad(mb)
  sqx/sqy = ACT Square; mag = sqx + DMA-accumulate(sqy)
  orientation (division-free): t = gy/gx in (A,B)u(C,D) with C=-1/A, D=-1/B
     <=> min(q1, q2) < 0,  q1 ~ (1-Q)sqy - P gxy + Q mag,
                           q2 ~ (Q-1)sqy + P gxy + mag   (P=A+B, Q=AB)
  NMS: ul = mag[r-1,c-1], dr = mag[r+1,c+1] via shift-banded fp32 matmuls
  mstar = oriented ? mag : 1e30 ; keep3 = ul < mstar ; keep7 = dr < mstar
  out = mag * keep3 * keep7

Row axis: 9 overlapping slabs of 128 partitions, 122 core rows each, 3 halo
rows per side; all cross-partition work happens inside per-slab banded
matmuls whose band matrices (built host-side from the actual input kernel
values) fold in jnp.pad(mode='edge') clamping. Column axis: 3-col pads on
each side of SBUF tiles, refreshed with edge values between stages.
"""

import math
import numpy as np

B, C, H, W = 8, 3, 1024, 1024
NCORES = 8
SLAB = 122
NSLABS = (H + SLAB - 1) // SLAB          # 9
PADL = 3                                  # col c stored at f = c + 3
FW = W + 2 * PADL                         # 1030
CHUNK = 512
BIG = np.float32(1e30)

A_ = -math.tan(3 * math.pi / 16)
B_ = -math.tan(math.pi / 16)
P_ = A_ + B_
Q_ = A_ * B_

_CACHE = {}


# ---------------------------------------------------------------------------
def _install_fixups():
    """This container's walrus encodes at most ONE sem wait per instruction
    (2 for EventSemaphore); the bass/tile build attaches more. Two patches:
    the TileContext tail drain (waits on every proc's clock) is split into a
    chain of single-wait sync nops, and a post-schedule pass moves excess
    waits from any instruction onto injected same-engine NoOps."""
    import concourse.tile as _tile
    from concourse.vector_clock import ScopedClock, VectorClock

    if getattr(_tile.TileContext, "_canny_patched", False):
        return

    def _drain_and_barrier(self, tick_clock, wait_clock):
        gcl = tick_clock.global_clock
        for i in range(len(gcl)):
            if gcl[i] == 0:
                continue
            vec = [0] * len(gcl)
            vec[i] = gcl[i]
            nop = self.nc.sync.nop(nofuse=True, hint="tail_drain_split")
            wait_clock.add_sem_waits(nop.ins,
                                     ScopedClock({None: VectorClock(vec)}))
        self.nc.sync.drain()
        self.nc.all_engine_barrier()
        assert self.sems is not None
        popped = self.nc._tile_sem_poison_stack.pop()
        assert popped is self._sem_poison
        self.nc.clear_and_free_semaphores(list(self.sems.allocated().values()))
        self.nc.all_engine_barrier()

    _tile.TileContext._drain_and_barrier = _drain_and_barrier
    _tile.TileContext._canny_patched = True


def _split_excess_waits(nc):
    import concourse.mybir as mybir
    for fn in nc.m.functions:
        for blk in fn.blocks:
            insts = list(blk.instructions)
            out, changed = [], False
            for inst in insts:
                si = inst.sync_info
                cap = 2 if isinstance(inst, mybir.InstEventSemaphore) else 1
                if si is not None and si.on_wait and len(si.on_wait) > cap:
                    waits = list(si.on_wait)
                    for j, wt in enumerate(waits[cap:]):
                        nop = mybir.InstNoOp(name=f"{inst.name}-wsplit{j}")
                        nop.engine = inst.engine
                        nop.sync_info = mybir.SyncInfo(on_wait=[wt],
                                                       on_update=[])
                        out.append(nop)
                    si.on_wait = waits[:cap]
                    inst.sync_info = si
                    changed = True
                out.append(inst)
            if changed:
                blk.instructions = out


# ---------------------------------------------------------------------------
# host-side band-matrix construction
def _r0(s):
    return SLAB * s - PADL


def _band(s, taps, clamp):
    """lhsT[k, m]: out[m] = sum_j taps[j] * in[k(m, j)] for slab s.
    k(m, j) = m + j, optionally clamped (in partition space) to the image
    edge partitions; unclamped out-of-range taps are dropped (those output
    rows are never consumed)."""
    Wm = np.zeros((128, 128), np.float64)
    lo = PADL if (clamp and s == 0) else None
    hi = (H - 1 - _r0(s)) if (clamp and s == NSLABS - 1) else None
    for m in range(128):
        for off, cf in taps.items():
            k = m + off
            if lo is not None and k < lo:
                k = lo
            if hi is not None and k > hi:
                k = hi
            if 0 <= k < 128:
                Wm[k, m] += cf
    return Wm


def _col_taps(k3x3, dc):
    col = k3x3[:, dc]
    return {j - 1: col[j] for j in range(3)}


def _bf16(x):
    import ml_dtypes
    return np.asarray(x, np.float32).astype(ml_dtypes.bfloat16)


def _build_bands(gauss_w, sobel_x, sobel_y, dir_w):
    """Returns (wb bf16 [nb,128,128], wf f32 [nf,128,128], index, meta).

    index maps (kind, variant_or_slabclass, dc, term) -> (which, idx).
    Variants v: 0 = top slab, 1 = interior, 2 = bottom slab.
    """
    import ml_dtypes
    g = np.asarray(gauss_w, np.float64).reshape(3, 3) / 3.0
    sx = np.asarray(sobel_x, np.float64).reshape(3, 3)
    sy = np.asarray(sobel_y, np.float64).reshape(3, 3)
    dw = np.asarray(dir_w, np.float64).reshape(8, 3, 3)

    wb, wf, index = [], [], {}

    def addb(key, mat64):
        hi = _bf16(mat64)
        lo64 = mat64 - hi.astype(np.float64)
        index[key] = ("b", len(wb))
        wb.append(hi)
        if np.abs(lo64).max() > 0:
            index[key + ("lo",)] = ("b", len(wb))
            wb.append(_bf16(lo64))

    def addf(key, mat64):
        index[key] = ("f", len(wf))
        wf.append(np.asarray(mat64, np.float32))

    sv = {0: 0, 1: 500, 2: NSLABS - 1}   # representative slab per variant
    for v in range(3):
        s = {0: 0, 1: 4, 2: NSLABS - 1}[v]
        if v == 1 or True:
            pass
        # blur: no clamping (input m already carries duplicated edge rows)
        if v == 1:
            for dc in range(3):
                addb(("blur", dc), _band(s, _col_taps(g, dc), clamp=False))
        # sobel: clamp to image-edge partitions
        for nm, kk in (("gx", sx), ("gy", sy)):
            for dc in range(3):
                if not np.any(kk[:, dc]):
                    continue
                addb((nm, v, dc), _band(s, _col_taps(kk, dc), clamp=True))
        # NMS shifted-copy bands from dir_w channels 3 and 7.
        # d3 = conv(pad(mag), dw3) = sum dc band; is_max needs d3>0 & d7>0.
        # We realize ul' = mag - d3 (so keep3 = ul' < mag ... ) -- simpler:
        # shift bands: for the reference kernels dw3 = center(+1)@(1,1),
        # corner(-1)@(0,0): mag - d3 = mag[r-1,c-1]. Generally:
        # ul = mag - d3 = conv(pad(mag), delta - dw3): build from that.
        delta = np.zeros((3, 3))
        delta[1, 1] = 1.0
        for nm, ch in (("ul", 3), ("dr", 7)):
            kk = delta - dw[ch]
            for dc in range(3):
                if not np.any(kk[:, dc]):
                    continue
                addf((nm, v, dc), _band(s, _col_taps(kk, dc), clamp=True))

    index[("nbig",)] = ("b", len(wb))
    wb.append(_bf16(-np.eye(128) * 1e30))
    wb = np.stack(wb).astype(ml_dtypes.bfloat16) if wb else \
        np.zeros((1, 128, 128), ml_dtypes.bfloat16)
    wf = np.stack(wf).astype(np.float32) if wf else \
        np.zeros((1, 128, 128), np.float32)
    return wb, wf, index


def _structure_key(index):
    return tuple(sorted(map(repr, index.keys())))


# ---------------------------------------------------------------------------
def _build_module(index, nb, nf):
    import concourse.bass as bass
    import concourse.tile as tile
    import concourse.mybir as mybir
    from contextlib import ExitStack

    F32 = mybir.dt.float32
    BF16 = mybir.dt.bfloat16
    AF = mybir.ActivationFunctionType
    Al = mybir.AluOpType

    nc = bass.Bass("TRN2", target_bir_lowering=False, debug=False,
                   num_devices=NCORES)
    img_d = nc.dram_tensor("img", [C, H, W], F32, kind="ExternalInput").ap()
    wb_d = nc.dram_tensor("wb", [nb, 128, 128], BF16,
                          kind="ExternalInput").ap()
    wf_d = nc.dram_tensor("wf", [nf, 128, 128], F32,
                          kind="ExternalInput").ap()
    out_d = nc.dram_tensor("out", [H, W], F32, kind="ExternalOutput").ap()

    def wm(key):
        which, i = index[key]
        return (wbt if which == "b" else wft)[:, i * 128:(i + 1) * 128]

    def has(key):
        return key in index

    with tile.TileContext(nc) as tc, ExitStack() as ctx:
        wpool = ctx.enter_context(tc.tile_pool(name="wpool", bufs=1))
        mpool = ctx.enter_context(tc.tile_pool(name="mpool", bufs=3))
        prp = ctx.enter_context(tc.tile_pool(name="prp", bufs=3))
        mbp = ctx.enter_context(tc.tile_pool(name="mbp", bufs=3))
        magp = ctx.enter_context(tc.tile_pool(name="magp", bufs=3))
        tl = ctx.enter_context(tc.tile_pool(name="tl", bufs=2))
        ps_mb = ctx.enter_context(
            tc.tile_pool(name="ps_mb", bufs=2, space="PSUM"))
        ps_g = ctx.enter_context(
            tc.tile_pool(name="ps_g", bufs=4, space="PSUM"))
        ps_e = ctx.enter_context(
            tc.tile_pool(name="ps_e", bufs=2, space="PSUM"))

        wbt = wpool.tile([128, nb * 128], BF16, name="wbt")
        for i in range(nb):
            nc.sync.dma_start(wbt[:, i * 128:(i + 1) * 128], wb_d[i])
        wft = wpool.tile([128, nf * 128], F32, name="wft")
        for i in range(nf):
            nc.sync.dma_start(wft[:, i * 128:(i + 1) * 128], wf_d[i])

        for s in range(NSLABS):
            v = 0 if s == 0 else (2 if s == NSLABS - 1 else 1)
            r0 = _r0(s)
            p_lo = PADL if s == 0 else 0
            p_hi = (H - 1 - r0) if s == NSLABS - 1 else 127

            # ---- load + channel sum ----
            m = mpool.tile([128, FW], F32, name="m", tag="m")
            if s == 0:
                nc.vector.memset(m[0:32, :], 0.0)
            if s == NSLABS - 1:
                nc.vector.memset(m[64:128, :], 0.0)
            for c in range(C):
                nc.gpsimd.dma_start(
                    m[p_lo:p_hi + 1, PADL:PADL + W],
                    img_d[c, r0 + p_lo:r0 + p_hi + 1, :],
                    accum_op=(Al.bypass if c == 0 else Al.add))
            if s == 0:                      # duplicated top edge row
                for c in range(C):
                    nc.gpsimd.dma_start(
                        m[PADL - 1:PADL, PADL:PADL + W], img_d[c, 0:1, :],
                        accum_op=(Al.bypass if c == 0 else Al.add))
            if s == NSLABS - 1:             # duplicated bottom edge row
                for c in range(C):
                    nc.gpsimd.dma_start(
                        m[p_hi + 1:p_hi + 2, PADL:PADL + W],
                        img_d[c, H - 1:H, :],
                        accum_op=(Al.bypass if c == 0 else Al.add))
            # col edge pads (both columns in one strided op, on ACT)
            nc.scalar.activation(m[:, PADL - 1:PADL + W + 1:W + 1],
                                 m[:, PADL:PADL + W:W - 1], AF.Copy)

            # ---- bf16 pair of m ----
            mhi = prp.tile([128, FW], BF16, name="mhi", tag="mhi")
            nc.scalar.activation(mhi[:], m[:], AF.Copy)
            mlo = prp.tile([128, FW], BF16, name="mlo", tag="mlo")
            nc.vector.scalar_tensor_tensor(
                mlo[:], mhi[:], -1.0, m[:], Al.mult, Al.add)

            # ---- blur ----
            mbhi = mbp.tile([128, FW], BF16, name="mbhi", tag="mbhi")
            mblo = mbp.tile([128, FW], BF16, name="mblo", tag="mblo")
            for h in range(2):
                f0 = PADL + CHUNK * h
                pm = ps_mb.tile([128, CHUNK], F32, name="pm", tag="pm")
                mms = []
                for dc in range(3):
                    mms.append((("blur", dc), mhi))
                    mms.append((("blur", dc), mlo))
                    if has(("blur", dc, "lo")):
                        mms.append((("blur", dc, "lo"), mhi))
                for i, (key, rhs) in enumerate(mms):
                    dc = key[1]
                    nc.tensor.matmul(
                        pm[:], wm(key),
                        rhs[:, f0 + dc - 1:f0 + dc - 1 + CHUNK],
                        start=(i == 0), stop=(i == len(mms) - 1))
                nc.scalar.activation(mbhi[:, f0:f0 + CHUNK], pm[:], AF.Copy)
                nc.vector.scalar_tensor_tensor(
                    mblo[:, f0:f0 + CHUNK], mbhi[:, f0:f0 + CHUNK], -1.0,
                    pm[:], Al.mult, Al.add)
            for t_ in (mbhi, mblo):
                nc.scalar.activation(t_[:, PADL - 1:PADL + W + 1:W + 1],
                                     t_[:, PADL:PADL + W:W - 1], AF.Copy)

            # ---- sobel + tail ----
            mag = magp.tile([128, FW], F32, name="mag", tag="mag")
            sqy = tl.tile([128, W], F32, name="sqy", tag="sqy")
            gys = tl.tile([128, W], F32, name="gys", tag="gys")
            gxy = tl.tile([128, W], F32, name="gxy", tag="gxy")
            t1 = tl.tile([128, W], F32, name="t1", tag="t1")
            q1 = tl.tile([128, W], F32, name="q1", tag="q1")
            t2 = tl.tile([128, W], F32, name="t2", tag="t2")
            q2 = tl.tile([128, W], F32, name="q2", tag="q2")
            vmin = tl.tile([128, W], F32, name="vmin", tag="vmin")
            nm = tl.tile([128, W], BF16, name="nm", tag="nm")
            for h in range(2):
                f0 = PADL + CHUNK * h
                w0 = CHUNK * h
                pgx = ps_g.tile([128, CHUNK], F32, name="pgx", tag="pg")
                mms = [(("gx", v, dc), rhs)
                       for dc in range(3) if has(("gx", v, dc))
                       for rhs in (mbhi, mblo)]
                for i, (key, rhs) in enumerate(mms):
                    dc = key[2]
                    nc.tensor.matmul(
                        pgx[:], wm(key),
                        rhs[:, f0 + dc - 1:f0 + dc - 1 + CHUNK],
                        start=(i == 0), stop=(i == len(mms) - 1))
                pgy = ps_g.tile([128, CHUNK], F32, name="pgy", tag="pg")
                mms = [(("gy", v, dc), rhs)
                       for dc in range(3) if has(("gy", v, dc))
                       for rhs in (mbhi, mblo)]
                for i, (key, rhs) in enumerate(mms):
                    dc = key[2]
                    nc.tensor.matmul(
                        pgy[:], wm(key),
                        rhs[:, f0 + dc - 1:f0 + dc - 1 + CHUNK],
                        start=(i == 0), stop=(i == len(mms) - 1))
                # sqx straight into mag; sqy + gy evac; gxy
                nc.scalar.activation(mag[:, f0:f0 + CHUNK], pgx[:], AF.Square)
                nc.scalar.activation(sqy[:, w0:w0 + CHUNK], pgy[:], AF.Square)
                nc.scalar.activation(gys[:, w0:w0 + CHUNK], pgy[:], AF.Copy)
                nc.vector.tensor_tensor(
                    gxy[:, w0:w0 + CHUNK], pgx[:], gys[:, w0:w0 + CHUNK],
                    Al.mult)
            # mag = sqx + sqy via SBUF->SBUF DMA accumulate
            nc.gpsimd.dma_start(mag[:, PADL:PADL + W], sqy[:],
                                accum_op=Al.add)
            nc.scalar.activation(mag[:, PADL - 1:PADL + W + 1:W + 1],
                                 mag[:, PADL:PADL + W:W - 1], AF.Copy)
            # orientation quadratics (sign-equivalent scalings)
            # q1 ~ mag - (P/Q) gxy + ((1-Q)/Q) sqy
            nc.vector.scalar_tensor_tensor(
                t1[:], gxy[:], -P_ / Q_, mag[:, PADL:PADL + W],
                Al.mult, Al.add)
            nc.vector.scalar_tensor_tensor(
                q1[:], sqy[:], (1.0 - Q_) / Q_, t1[:], Al.mult, Al.add)
            # q2 = (Q-1) sqy + P gxy + mag; adds on GPSIMD (only add/mult
            # have Q7 impls), scale factors pre-applied on DVE tensor_scalar
            gxy3 = tl.tile([128, W], F32, name="gxy3", tag="gxy3")
            sqy2 = tl.tile([128, W], F32, name="sqy2", tag="sqy2")
            nc.vector.tensor_scalar(gxy3[:], gxy[:], P_, None, Al.mult)
            nc.vector.tensor_scalar(sqy2[:], sqy[:], Q_ - 1.0, None, Al.mult)
            nc.gpsimd.tensor_tensor(t2[:], gxy3[:], mag[:, PADL:PADL + W],
                                    Al.add)
            nc.gpsimd.tensor_tensor(q2[:], sqy2[:], t2[:], Al.add)
            nc.vector.tensor_tensor(vmin[:], q1[:], q2[:], Al.min)
            # nm = 1 where NOT oriented (bf16; feeds the -BIG*nm matmul term)
            nc.vector.tensor_scalar(nm[:], vmin[:], 0.0, None, Al.is_ge)

            # ---- NMS shifts (fp32 matmuls) + keep + final ----
            k3 = tl.tile([128, W], F32, name="k3", tag="k3")
            k7 = tl.tile([128, W], F32, name="k7", tag="k7")
            o1 = tl.tile([128, W], F32, name="o1", tag="o1")
            fin = tl.tile([128, W], F32, name="fin", tag="fin")
            for h in range(2):
                f0 = PADL + CHUNK * h
                w0 = CHUNK * h
                pul = ps_e.tile([128, CHUNK], F32, name="pul", tag="pe")
                mms = [("ul", v, dc) for dc in range(3)
                       if has(("ul", v, dc))]
                for i, key in enumerate(mms):
                    dc = key[2]
                    nc.tensor.matmul(
                        pul[:], wm(key),
                        mag[:, f0 + dc - 1:f0 + dc - 1 + CHUNK],
                        start=(i == 0), stop=False)
                nc.tensor.matmul(pul[:], wm(("nbig",)),
                                 nm[:, w0:w0 + CHUNK],
                                 start=False, stop=True)
                nc.vector.tensor_tensor(
                    k3[:, w0:w0 + CHUNK], pul[:],
                    mag[:, f0:f0 + CHUNK], Al.is_lt)
                pdr = ps_e.tile([128, CHUNK], F32, name="pdr", tag="pe")
                mms = [("dr", v, dc) for dc in range(3)
                       if has(("dr", v, dc))]
                for i, key in enumerate(mms):
                    dc = key[2]
                    nc.tensor.matmul(
                        pdr[:], wm(key),
                        mag[:, f0 + dc - 1:f0 + dc - 1 + CHUNK],
                        start=(i == 0), stop=False)
                nc.tensor.matmul(pdr[:], wm(("nbig",)),
                                 nm[:, w0:w0 + CHUNK],
                                 start=False, stop=True)
                nc.vector.tensor_tensor(
                    k7[:, w0:w0 + CHUNK], pdr[:],
                    mag[:, f0:f0 + CHUNK], Al.is_lt)
                nc.gpsimd.tensor_tensor(
                    o1[:, w0:w0 + CHUNK], mag[:, f0:f0 + CHUNK],
                    k3[:, w0:w0 + CHUNK], Al.mult)
                nc.vector.tensor_tensor(
                    fin[:, w0:w0 + CHUNK], o1[:, w0:w0 + CHUNK],
                    k7[:, w0:w0 + CHUNK], Al.mult)

            row_lo = SLAB * s
            row_hi = min(H - 1, row_lo + SLAB - 1)
            nc.sync.dma_start(out_d[row_lo:row_hi + 1, :],
                              fin[PADL:PADL + row_hi - row_lo + 1, :])

    _split_excess_waits(nc)
    return nc


# ---------------------------------------------------------------------------
def kernel(**inputs):
    _install_fixups()

    img = np.ascontiguousarray(np.asarray(inputs["img"], np.float32))
    gauss_w = np.asarray(inputs["gauss_w"], np.float32)
    sobel_x = np.asarray(inputs["sobel_x"], np.float32)
    sobel_y = np.asarray(inputs["sobel_y"], np.float32)
    dir_w = np.asarray(inputs["dir_w"], np.float32)

    wb, wf, index = _build_bands(gauss_w, sobel_x, sobel_y, dir_w)
    skey = _structure_key(index)
    if _CACHE.get("skey") != skey:
        _CACHE["nc"] = _build_module(index, wb.shape[0], wf.shape[0])
        _CACHE["skey"] = skey
    nc = _CACHE["nc"]

    from concourse.bass_utils import run_bass_kernel_spmd
    import os
    wb = np.ascontiguousarray(wb)
    wf = np.ascontiguousarray(wf)
    in_maps = [{"img": np.ascontiguousarray(img[b]), "wb": wb, "wf": wf}
               for b in range(B)]
    trace = bool(int(os.environ.get("CANNY_TRACE", "0")))
    res = run_bass_kernel_spmd(nc, in_maps, core_ids=list(range(NCORES)),
                               trace=trace)
    if res.exec_time_ns is not None:
        _CACHE["exec_time_ns"] = res.exec_time_ns
    if res.instructions_and_trace is not None:
        _CACHE["trace_path"] = res.instructions_and_trace[1]
    out = np.stack([res.results[b]["out"] for b in range(B)])[:, None]
    return out.astype(np.float32)



# revision 31
# speedup vs baseline: 1.5850x; 1.5850x over previous
"""Canny NMS filter for 8x Trainium2 NeuronCores (Bass/Tile).

Batch is sharded one image per core. Per core (img 3x1024x1024 -> 1024x1024):

  m   = (c0+c1+c2)             c0 via HWDGE, c1/c2 DMA-accumulate (SWDGE)
  mb  = gauss3x3 (x) pad(m)    3 banded fp32r matmuls per 512-chunk
  gx  = sobelx (x) pad(mb)     2 fp32r matmuls ; gy = sobely: 3 fp32r matmuls
  sqx/sqy = ACT Square; mag = sqx+sqy
  orientation (division-free): t = gy/gx in (A,B)u(-1/A,-1/B)
     <=> u*v < 0 with u = ka*(sqy-sqx) + gxy, v = kb*(sqy-sqx) + gxy,
     ka = A/(1-A^2), kb = B/(1-B^2)   (positive rescalings of the two
     quadratics (gy-Agx)(Agy+gx), (gy-Bgx)(Bgy+gx))
  NMS: d3 = mag - mag[r-1,c-1], d7 = mag - mag[r+1,c+1] as 2-matmul PSUM
     groups (center identity band + shifted band from the dir_w taps)
  keep = max(min(d3,d7), u*v) >= 0 ; out = mag*keep
     (sign-only quantities u,v,om,vm,mx are stored bf16: fp32->bf16
     rounding never flips a sign, and all-bf16 SBUF TensorScalarPtr ops
     run in the 4x DVE perf mode)

Row axis: 9 overlapping slabs of 128 partitions, 122 core rows each, 3 halo
rows per side; cross-partition work happens inside per-slab banded matmuls
whose band matrices (built host-side from the actual kernel values) fold in
jnp.pad(mode='edge') clamping at every stage. Column axis: 1-col edge pads
refreshed per stage. All matmul moving operands are bitcast to float32r
(full-rate on the PE at free-size >= 256, exact fp32 accumulate).
"""

import math
import numpy as np

B, C, H, W = 8, 3, 1024, 1024
NCORES = 8
SLAB = 122
NSLABS = (H + SLAB - 1) // SLAB          # 9
HALO = 3                                  # row halo (partition space)
PADL = 1                                  # col c stored at f = c + 1
FW = W + 2 * PADL                         # 1026
CHUNK = 512

A_ = -math.tan(3 * math.pi / 16)
B_ = -math.tan(math.pi / 16)
KA = A_ / (1.0 - A_ * A_)
KB = B_ / (1.0 - B_ * B_)

_CACHE = {}


# ---------------------------------------------------------------------------
def _install_fixups():
    """This container's walrus encodes at most ONE sem wait per instruction
    (2 for EventSemaphore); the bass/tile build attaches more. Two patches:
    the TileContext tail drain (waits on every proc's clock) is split into a
    chain of single-wait sync nops, and a post-schedule pass moves excess
    waits from any instruction onto injected same-engine NoOps."""
    import concourse.tile as _tile
    from concourse.vector_clock import ScopedClock, VectorClock

    if getattr(_tile.TileContext, "_canny_patched", False):
        return

    def _drain_and_barrier(self, tick_clock, wait_clock):
        gcl = tick_clock.global_clock
        for i in range(len(gcl)):
            if gcl[i] == 0:
                continue
            vec = [0] * len(gcl)
            vec[i] = gcl[i]
            nop = self.nc.sync.nop(nofuse=True, hint="tail_drain_split")
            wait_clock.add_sem_waits(nop.ins,
                                     ScopedClock({None: VectorClock(vec)}))
        self.nc.sync.drain()
        self.nc.all_engine_barrier()
        assert self.sems is not None
        popped = self.nc._tile_sem_poison_stack.pop()
        assert popped is self._sem_poison
        self.nc.clear_and_free_semaphores(list(self.sems.allocated().values()))
        self.nc.all_engine_barrier()

    _tile.TileContext._drain_and_barrier = _drain_and_barrier
    _tile.TileContext._canny_patched = True


def _split_excess_waits(nc):
    import concourse.mybir as mybir
    for fn in nc.m.functions:
        for blk in fn.blocks:
            insts = list(blk.instructions)
            out, changed = [], False
            for inst in insts:
                si = inst.sync_info
                cap = 2 if isinstance(inst, mybir.InstEventSemaphore) else 1
                if si is not None and si.on_wait and len(si.on_wait) > cap:
                    waits = list(si.on_wait)
                    for j, wt in enumerate(waits[cap:]):
                        nop = mybir.InstNoOp(name=f"{inst.name}-wsplit{j}")
                        nop.engine = inst.engine
                        nop.sync_info = mybir.SyncInfo(on_wait=[wt],
                                                       on_update=[])
                        out.append(nop)
                    si.on_wait = waits[:cap]
                    inst.sync_info = si
                    changed = True
                out.append(inst)
            if changed:
                blk.instructions = out


# ---------------------------------------------------------------------------
# host-side band-matrix construction
def _r0(s):
    return SLAB * s - HALO


def _band(s, taps, clamp):
    """lhsT[k, m]: out[m] = sum_j taps[j] * in[k(m, j)] for slab s.
    k(m, j) = m + j, optionally clamped (in partition space) to the image
    edge partitions; unclamped out-of-range taps are dropped (those output
    rows are never consumed)."""
    Wm = np.zeros((128, 128), np.float64)
    lo = HALO if (clamp and s == 0) else None
    hi = (H - 1 - _r0(s)) if (clamp and s == NSLABS - 1) else None
    for m in range(128):
        for off, cf in taps.items():
            k = m + off
            if lo is not None and k < lo:
                k = lo
            if hi is not None and k > hi:
                k = hi
            if 0 <= k < 128:
                Wm[k, m] += cf
    return Wm


def _col_taps(k3x3, dc):
    col = k3x3[:, dc]
    return {j - 1: col[j] for j in range(3)}


def _build_bands(gauss_w, sobel_x, sobel_y, dir_w):
    """Returns (wf f32 [128, nf*128], findex).

    Band matrices (all fp32; bitcast to float32r at use so both matmul
    operands share a transfer type) are packed pre-transposed,
    [k, i*128 + m], so the whole set loads with one DMA of 128
    full-partition-contiguous descriptors. Identical matrices are
    deduplicated. Variants v: 0 = top slab, 1 = interior, 2 = bottom
    slab; every band clamps its taps to the image edge rows (replicate
    padding)."""
    g = np.asarray(gauss_w, np.float64).reshape(3, 3) / 3.0
    sx = np.asarray(sobel_x, np.float64).reshape(3, 3)
    sy = np.asarray(sobel_y, np.float64).reshape(3, 3)
    dw = np.asarray(dir_w, np.float64).reshape(8, 3, 3)

    wf, findex, seen = [], {}, {}

    def add(key, mat):
        b = mat.tobytes()
        if b not in seen:
            seen[b] = len(wf)
            wf.append(mat)
        findex[key] = seen[b]

    sv = {0: 0, 1: 4, 2: NSLABS - 1}
    for v in range(3):
        s = sv[v]
        for nm, kk in (("blur", g), ("gx", sx), ("gy", sy),
                       ("n3", dw[3]), ("n7", dw[7])):
            for dc in range(3):
                if np.any(kk[:, dc]):
                    add((nm, v, dc), _band(s, _col_taps(kk, dc), clamp=True))
    add(("id",), np.eye(128))

    wf_t = np.stack(wf).transpose(1, 0, 2).reshape(128, -1)
    return np.ascontiguousarray(wf_t.astype(np.float32)), findex


def _structure_key(findex):
    return tuple(sorted(map(repr, findex.keys())))


# ---------------------------------------------------------------------------
def _build_module(findex, nf):
    import concourse.bass as bass
    import concourse.tile as tile
    import concourse.mybir as mybir
    from contextlib import ExitStack

    F32 = mybir.dt.float32
    F32R = mybir.dt.float32r
    BF16 = mybir.dt.bfloat16
    AF = mybir.ActivationFunctionType
    Al = mybir.AluOpType

    nc = bass.Bass("TRN2", target_bir_lowering=False, debug=False,
                   num_devices=NCORES)
    img_d = nc.dram_tensor("img", [C, H, W], F32, kind="ExternalInput").ap()
    wf_d = nc.dram_tensor("wf", [128, nf * 128], F32R,
                          kind="ExternalInput").ap()
    out_d = nc.dram_tensor("out", [H, W], F32, kind="ExternalOutput").ap()

    with tile.TileContext(nc) as tc, ExitStack() as ctx:
        wpool = ctx.enter_context(tc.tile_pool(name="wpool", bufs=1))
        s0p = ctx.enter_context(tc.tile_pool(name="s0p", bufs=1))
        mpool = ctx.enter_context(tc.tile_pool(name="mpool", bufs=3))
        mbp = ctx.enter_context(tc.tile_pool(name="mbp", bufs=3))
        magp = ctx.enter_context(tc.tile_pool(name="magp", bufs=4))
        tl = ctx.enter_context(tc.tile_pool(name="tl", bufs=3))
        ps_b = ctx.enter_context(
            tc.tile_pool(name="ps_b", bufs=2, space="PSUM"))
        ps_g = ctx.enter_context(
            tc.tile_pool(name="ps_g", bufs=3, space="PSUM"))
        ps_m = ctx.enter_context(
            tc.tile_pool(name="ps_m", bufs=1, space="PSUM"))
        ps_e = ctx.enter_context(
            tc.tile_pool(name="ps_e", bufs=2, space="PSUM"))

        wft = wpool.tile([128, nf * 128], F32R, name="wft")

        def wmf(key):
            i = findex[key]
            return wft[:, i * 128:(i + 1) * 128].bitcast(F32R)

        def mm_group(psum, group, rhs_tile, f0):
            for i, key in enumerate(group):
                dc = key[2] if len(key) > 2 else 1
                nc.tensor.matmul(
                    psum[:], wmf(key),
                    rhs_tile[:, f0 + dc - 1:f0 + dc - 1 + CHUNK]
                    .bitcast(F32R),
                    start=(i == 0), stop=(i == len(group) - 1))

        mt = {}      # per-slab live tiles

        def emit_loads(s):
            """m tile + channel-summed image loads for slab s (c0 on the SP
            HWDGE queue, c1/c2 Pool DMA-accumulate). Slab 0 loads in column
            halves so its first blur chunk starts as early as possible, with
            the weight DMAs slotted behind the left half."""
            r0 = _r0(s)
            p_lo = HALO if s == 0 else 0
            p_hi = (H - 1 - r0) if s == NSLABS - 1 else 127
            m = mpool.tile([128, FW], F32R, name="m", tag="m")
            if s == 0:
                nc.gpsimd.memset(m[0:32, :].bitcast(F32), 0.0)
            if s == NSLABS - 1:
                # partition windows not starting at 0 are limited to 32
                # partitions; zero the whole tile before the row loads land
                nc.gpsimd.memset(m[:, :].bitcast(F32), 0.0)
            rows = img_d[:, r0 + p_lo:r0 + p_hi + 1, :]
            if s == 0:
                # Parallel per-channel tiles: no serial DMA-accumulate chain
                # on the critical path; the channel sum rides the blur's PSUM
                # accumulation (9 matmuls instead of 3, once).
                nc.sync.dma_start(wft[:], wf_d[:])
                mlist = [m,
                         s0p.tile([128, FW], F32R, name="mB"),
                         s0p.tile([128, FW], F32R, name="mC")]
                for c, mc in enumerate(mlist):
                    if c > 0:
                        nc.gpsimd.memset(mc[0:32, :].bitcast(F32), 0.0)
                    nc.sync.dma_start(mc[p_lo:p_hi + 1, PADL:PADL + W],
                                      rows[c].bitcast(F32R))
                for mc in mlist:
                    nc.scalar.activation(mc[:, 0:FW:FW - 1],
                                         mc[:, PADL:PADL + W:W - 1], AF.Copy)
                mt[s] = {"mlist": mlist}
            else:
                nc.sync.dma_start(m[p_lo:p_hi + 1, PADL:PADL + W],
                                  rows[0].bitcast(F32R))
                for c in range(1, C):
                    nc.gpsimd.dma_start(m[p_lo:p_hi + 1, PADL:PADL + W],
                                        rows[c], accum_op=Al.add)
                mt[s] = {"mlist": [m]}

        def stage_a(s):
            """blur + sobel + squares/gxy for slab s."""
            v = 0 if s == 0 else (2 if s == NSLABS - 1 else 1)
            t = mt[s]
            mlist = t["mlist"]
            if s != 0:          # slab 0's pads were emitted with its loads
                nc.scalar.activation(mlist[0][:, 0:FW:FW - 1],
                                     mlist[0][:, PADL:PADL + W:W - 1],
                                     AF.Copy)
            mb = mbp.tile([128, FW], F32R, name="mb", tag="mb")
            blur = [("blur", v, dc) for dc in range(3)
                    if ("blur", v, dc) in findex]
            for h in range(2):
                f0 = PADL + CHUNK * h
                pm = ps_b.tile([128, CHUNK], F32, name="pm", tag="pm")
                group = [(key, mc) for mc in mlist for key in blur]
                for i, (key, mc) in enumerate(group):
                    dc = key[2]
                    nc.tensor.matmul(
                        pm[:], wmf(key),
                        mc[:, f0 + dc - 1:f0 + dc - 1 + CHUNK].bitcast(F32R),
                        start=(i == 0), stop=(i == len(group) - 1))
                nc.scalar.activation(mb[:, f0:f0 + CHUNK], pm[:], AF.Copy)
                # left col pad after chunk 0, right col pad after chunk 1
                nc.scalar.activation(mb[:, h * (FW - 1):h * (FW - 1) + 1],
                                     mb[:, PADL + h * (W - 1):
                                         PADL + h * (W - 1) + 1], AF.Copy)
            sqx = tl.tile([128, W], F32R, name="sqx", tag="sqx")
            sqy = tl.tile([128, W], F32R, name="sqy", tag="sqy")
            gyv = tl.tile([128, W], F32, name="gyv", tag="gyv")
            gxy = tl.tile([128, W], F32, name="gxy", tag="gxy")
            gxk = [("gx", v, dc) for dc in range(3) if ("gx", v, dc) in findex]
            gyk = [("gy", v, dc) for dc in range(3) if ("gy", v, dc) in findex]
            for h in range(2):
                f0 = PADL + CHUNK * h
                w0 = CHUNK * h
                pgx = ps_g.tile([128, CHUNK], F32, name="pgx", tag="pg")
                mm_group(pgx, gxk, mb, f0)
                pgy = ps_g.tile([128, CHUNK], F32, name="pgy", tag="pg")
                mm_group(pgy, gyk, mb, f0)
                nc.scalar.activation(sqx[:, w0:w0 + CHUNK], pgx[:], AF.Square)
                nc.scalar.activation(sqy[:, w0:w0 + CHUNK], pgy[:], AF.Square)
                nc.scalar.activation(gyv[:, w0:w0 + CHUNK], pgy[:], AF.Copy)
                nc.vector.tensor_tensor(
                    gxy[:, w0:w0 + CHUNK], pgx[:], gyv[:, w0:w0 + CHUNK],
                    Al.mult)
            t.update(sqx=sqx, sqy=sqy, gxy=gxy)

        def stage_b(s):
            """mag (identity-band matmuls) + orientation for slab s."""
            t = mt[s]
            sqx, sqy, gxy = t["sqx"], t["sqy"], t["gxy"]
            mag = magp.tile([128, FW], F32R, name="mag", tag="mag")
            for h in range(2):
                f0 = PADL + CHUNK * h
                w0 = CHUNK * h
                pmag = ps_m.tile([128, CHUNK], F32, name="pmag", tag="pmag")
                nc.tensor.matmul(pmag[:], wmf(("id",)),
                                 sqx[:, w0:w0 + CHUNK].bitcast(F32R),
                                 start=True, stop=False)
                nc.tensor.matmul(pmag[:], wmf(("id",)),
                                 sqy[:, w0:w0 + CHUNK].bitcast(F32R),
                                 start=False, stop=True)
                nc.scalar.activation(mag[:, f0:f0 + CHUNK], pmag[:], AF.Copy)
                nc.scalar.activation(mag[:, h * (FW - 1):h * (FW - 1) + 1],
                                     mag[:, PADL + h * (W - 1):
                                         PADL + h * (W - 1) + 1], AF.Copy)
            d = tl.tile([128, W], F32, name="d", tag="d")
            u = tl.tile([128, W], BF16, name="u", tag="u")
            vt = tl.tile([128, W], BF16, name="vt", tag="vt")
            om = tl.tile([128, W], BF16, name="om", tag="om")
            nc.gpsimd.tensor_tensor(
                d[:], sqy[:].bitcast(F32), sqx[:].bitcast(F32), Al.subtract)
            nc.vector.scalar_tensor_tensor(
                u[:], d[:], KA, gxy[:], Al.mult, Al.add)
            nc.vector.scalar_tensor_tensor(
                vt[:], d[:], KB, gxy[:], Al.mult, Al.add)
            nc.vector.tensor_tensor(om[:], u[:], vt[:], Al.mult)
            t.update(mag=mag, om=om)

        def tail(s):
            """NMS + keep-mask + store for slab s. The last slab's combine +
            store run per column half on the (by then idle) DVE to shorten
            the serial endgame."""
            v = 0 if s == 0 else (2 if s == NSLABS - 1 else 1)
            last = s == NSLABS - 1
            t = mt.pop(s)
            mag, om = t["mag"], t["om"]
            vm = tl.tile([128, W], BF16, name="vm", tag="vm")
            mx = tl.tile([128, W], BF16, name="mx", tag="mx")
            kp = tl.tile([128, W], F32, name="kp", tag="kp")
            fin = tl.tile([128, W], F32, name="fin", tag="fin")
            n3k = [("n3", v, dc) for dc in range(3) if ("n3", v, dc) in findex]
            n7k = [("n7", v, 2)]          # shift band only: p7 = -mag[r+1,c+1]
            e7 = tl.tile([128, W], F32, name="e7", tag="e7")
            row_lo = SLAB * s
            row_hi = min(H - 1, row_lo + SLAB - 1)
            nrow = row_hi - row_lo + 1
            for h in range(2):
                f0 = PADL + CHUNK * h
                w0 = CHUNK * h
                p3 = ps_e.tile([128, CHUNK], F32, name="p3", tag="pe")
                mm_group(p3, n3k, mag, f0)
                p7 = ps_e.tile([128, CHUNK], F32, name="p7", tag="pe")
                mm_group(p7, n7k, mag, f0)
                nc.vector.scalar_tensor_tensor(
                    e7[:, w0:w0 + CHUNK], p7[:], 1.0,
                    mag[:, f0:f0 + CHUNK].bitcast(F32), Al.mult, Al.add)
                nc.vector.scalar_tensor_tensor(
                    vm[:, w0:w0 + CHUNK], p3[:], 0.0, e7[:, w0:w0 + CHUNK],
                    Al.add, Al.min)
                if last:
                    nc.vector.tensor_tensor(mx[:, w0:w0 + CHUNK],
                                            vm[:, w0:w0 + CHUNK],
                                            om[:, w0:w0 + CHUNK], Al.max)
                    nc.vector.tensor_scalar(kp[:, w0:w0 + CHUNK],
                                            mx[:, w0:w0 + CHUNK], 0.0, None,
                                            Al.is_ge)
                    nc.vector.tensor_tensor(
                        fin[:, w0:w0 + CHUNK], kp[:, w0:w0 + CHUNK],
                        mag[:, f0:f0 + CHUNK].bitcast(F32), Al.mult)
                    nc.scalar.dma_start(
                        out_d[row_lo:row_hi + 1, w0:w0 + CHUNK],
                        fin[HALO:HALO + nrow, w0:w0 + CHUNK])
            if not last:
                nc.vector.tensor_tensor(mx[:], vm[:], om[:], Al.max)
                nc.vector.tensor_scalar(kp[:], mx[:], 0.0, None, Al.is_ge)
                nc.gpsimd.tensor_tensor(
                    fin[:], kp[:], mag[:, PADL:PADL + W].bitcast(F32), Al.mult)
                nc.scalar.dma_start(out_d[row_lo:row_hi + 1, :],
                                    fin[HALO:HALO + nrow, :])

        emit_loads(0)
        for s in range(NSLABS):
            if s + 1 < NSLABS:
                emit_loads(s + 1)
            stage_a(s)
            if s >= 1:
                stage_b(s - 1)
            if s >= 2:
                tail(s - 2)
        stage_b(NSLABS - 1)
        tail(NSLABS - 2)
        tail(NSLABS - 1)

    _split_excess_waits(nc)
    return nc


# ---------------------------------------------------------------------------
def kernel(**inputs):
    _install_fixups()

    img = np.ascontiguousarray(np.asarray(inputs["img"], np.float32))
    gauss_w = np.asarray(inputs["gauss_w"], np.float32)
    sobel_x = np.asarray(inputs["sobel_x"], np.float32)
    sobel_y = np.asarray(inputs["sobel_y"], np.float32)
    dir_w = np.asarray(inputs["dir_w"], np.float32)

    wf, findex = _build_bands(gauss_w, sobel_x, sobel_y, dir_w)
    skey = _structure_key(findex)
    if _CACHE.get("skey") != skey:
        _CACHE["nc"] = _build_module(findex, wf.shape[1] // 128)
        _CACHE["skey"] = skey
    nc = _CACHE["nc"]

    from concourse.bass_utils import run_bass_kernel_spmd
    import os
    in_maps = [{"img": np.ascontiguousarray(img[b]), "wf": wf}
               for b in range(B)]
    trace = bool(int(os.environ.get("CANNY_TRACE", "0")))
    res = run_bass_kernel_spmd(nc, in_maps, core_ids=list(range(NCORES)),
                               trace=trace)
    if res.exec_time_ns is not None:
        _CACHE["exec_time_ns"] = res.exec_time_ns
    if res.instructions_and_trace is not None:
        _CACHE["trace_path"] = res.instructions_and_trace[1]
    out = np.stack([res.results[b]["out"] for b in range(B)])[:, None]
    return out.astype(np.float32)
